# revision 28
# baseline (speedup 1.0000x reference)
"""BSM (bipartite soft matching) token-merge kernel for Trainium2.

Data-parallel over the batch dim: 64 batch rows are split 8-per-core
across 8 NeuronCores; each core runs an identical Bass program.

Emission is software-pipelined (payload lags index by LAG batches) so
the GPSIMD stream interleaves as ls(0..LAG-1), gather(0), ls(LAG),
gather(1), ... and payload gathers never queue behind index ops that
are still waiting on their inputs.

Per batch:
  index:   scores = a @ b.T (PE fp32); node_max/argmax (DVE fused
           reduces); rank via one-hot comparisons (PE block-diagonal
           broadcast + DVE); perm/dstv via GPSIMD local_scatter.
  payload: dma_gather src rows in rank order (merged/unmerged
           halves); unmerged stored verbatim (fp32); merged summed
           into dst tokens via bf16 one-hot matmul on the PE (one-hot
           side exact; payload side rounds to bf16 on the merged-sum
           term only), added to dst tokens in fp32.
"""

import sys
from contextlib import ExitStack

for _p in ("/root/.axon_site/_ro/trn_rl_repo", "/opt/trn_rl_repo"):
    if _p not in sys.path:
        sys.path.append(_p)

import numpy as np  # noqa: E402

from concourse import bacc, bass, tile  # noqa: E402
from concourse import mybir  # noqa: E402
from concourse.bass_utils import run_bass_kernel_spmd  # noqa: E402

DT = mybir.dt
F32 = DT.float32
BF16 = DT.bfloat16
F16 = DT.float16
I16 = DT.int16
ALU = mybir.AluOpType
AX = mybir.AxisListType

B, T, C, CK, R = 64, 1024, 768, 64, 256
NCORES = 8
BL = B // NCORES          # 8 batch rows per core
TH = T // 2               # 512 source (and dst) tokens
NU = TH - R               # 256 unmerged tokens
NCH = TH // 128           # 4 chunks of 128 source tokens

NEG_INF = -1e30
LAG = 5                   # payload emission lags index by LAG batches


def build_nc(bl: int = BL, debug: bool = False):
    nc = bacc.Bacc("TRN2", target_bir_lowering=False, debug=debug)
    x = nc.dram_tensor("x", [bl, T, C], F32, kind="ExternalInput")
    k = nc.dram_tensor("k", [bl, T, CK], F32, kind="ExternalInput")
    out = nc.dram_tensor("out", [bl, T - R, C], F32, kind="ExternalOutput")

    with tile.TileContext(nc) as tc:
        emit(tc, out.ap(), x.ap(), k.ap(), bl)

    nc.compile()
    return nc


class P:
    """Pool + constant handles shared by the emit phases."""


def emit(tc: tile.TileContext, out: bass.AP, x: bass.AP, k: bass.AP, bl: int):
    nc = tc.nc
    ctx = ExitStack()
    with ctx:
        p = P()
        p.const = ctx.enter_context(tc.tile_pool(name="const", bufs=1))
        p.kall = ctx.enter_context(tc.tile_pool(name="kall", bufs=1))
        p.kt = ctx.enter_context(tc.tile_pool(name="kt", bufs=3))
        p.small = ctx.enter_context(tc.tile_pool(name="small", bufs=3))
        p.scr = ctx.enter_context(tc.tile_pool(name="scr", bufs=3))
        p.idx = ctx.enter_context(tc.tile_pool(name="idx", bufs=7))
        p.mm = ctx.enter_context(tc.tile_pool(name="mm", bufs=2))
        p.ho = ctx.enter_context(tc.tile_pool(name="ho", bufs=LAG + 2))
        p.g = ctx.enter_context(tc.tile_pool(name="g", bufs=3))
        p.d = ctx.enter_context(tc.tile_pool(name="d", bufs=2))
        p.o = ctx.enter_context(tc.tile_pool(name="o", bufs=2))
        p.dram = ctx.enter_context(tc.tile_pool(name="dram", bufs=4,
                                                space="DRAM"))
        p.ps_score = ctx.enter_context(
            tc.tile_pool(name="ps_score", bufs=2, space="PSUM"))
        p.ps_tr = ctx.enter_context(
            tc.tile_pool(name="ps_tr", bufs=2, space="PSUM"))
        p.ps_vb = ctx.enter_context(
            tc.tile_pool(name="ps_vb", bufs=2, space="PSUM"))
        p.ps_s = ctx.enter_context(
            tc.tile_pool(name="ps_s", bufs=2, space="PSUM"))

        # ---- constants ----
        ones_sq = p.const.tile([128, 128], F32)
        nc.vector.memset(ones_sq[:], 1.0)
        p.ident = p.const.tile([128, 128], F32)    # PE transpose identity
        nc.gpsimd.affine_select(
            p.ident[:], ones_sq[:], pattern=[[-1, 128]], base=0,
            channel_multiplier=1, compare_op=ALU.is_equal, fill=0.0)
        p.ones4 = p.const.tile([4, 128], F32)      # lhsT for bcast matmul
        nc.vector.memset(p.ones4[:], 1.0)
        ones_row = p.const.tile([128, TH], F32)
        nc.vector.memset(ones_row[:], 1.0)
        p.iota_bc = p.const.tile([128, TH], F32)   # 0..511 per partition
        nc.gpsimd.iota(p.iota_bc[:], pattern=[[1, TH]], base=0,
                       channel_multiplier=0,
                       allow_small_or_imprecise_dtypes=True)
        p.iota_rev = p.const.tile([128, TH], F32)  # 511..0 per partition
        nc.gpsimd.iota(p.iota_rev[:], pattern=[[-1, TH]], base=TH - 1,
                       channel_multiplier=0,
                       allow_small_or_imprecise_dtypes=True)
        p.tri = []
        for mc in range(NCH):
            t_ = p.const.tile([128, TH], F32, tag=f"tri{mc}")
            # tri[i_local, j] = 1 if j < 128*mc + i_local else 0
            nc.gpsimd.affine_select(
                t_[:], ones_row[:], pattern=[[-1, TH]], base=128 * mc,
                channel_multiplier=1, compare_op=ALU.is_gt, fill=0.0)
            p.tri.append(t_)
        # blockmask[kb, j] = 1 if j // 128 == kb else 0 (rows 0..3)
        bm_iota = p.const.tile([4, TH], F32, tag="bm_iota")
        nc.gpsimd.iota(bm_iota[:], pattern=[[-1, NCH], [0, 128]],
                       base=0, channel_multiplier=1,
                       allow_small_or_imprecise_dtypes=True)
        p.blockmask = p.const.tile([4, TH], F32, tag="bm_sel")
        nc.vector.tensor_scalar(p.blockmask[:], bm_iota[:], 0.0, None,
                                op0=ALU.is_equal)
        # sigma-order token ids: data_iota[0, 4*p + c] = p + 128*c
        p.data_iota = p.const.tile([1, TH], I16)
        nc.gpsimd.iota(p.data_iota[:], pattern=[[1, 128], [128, NCH]],
                       base=0, channel_multiplier=0)
        p.iota16 = p.const.tile([128, TH], F16, tag="iota16")
        nc.gpsimd.iota(p.iota16[:], pattern=[[1, TH]], base=0,
                       channel_multiplier=0,
                       allow_small_or_imprecise_dtypes=True)

        # ---- bulk k load for all local batches ----
        # kall[p, h, b, mc, c] = k[b, 2*(mc*128+p)+h, c]
        p.kraw = p.kall.tile([128, 2, bl, NCH, CK], F32, tag="kraw")
        for h in range(2):
            k_src = bass.AP(k.tensor, k.offset + h * CK,
                            [[2 * CK, 128], [T * CK, bl],
                             [2 * 128 * CK, NCH], [1, CK]])
            nc.sync.dma_start(p.kraw[:, h, :, :, :], k_src)

        # ---- software-pipelined emission ----
        ho = {}
        for step in range(bl + LAG):
            if step < bl:
                ho[step] = _emit_index(tc, nc, step, p)
            if step >= LAG:
                b = step - LAG
                _emit_payload(tc, nc, out, x, b, p, *ho.pop(b))


def _emit_index(tc, nc, b, p):
    # ---- transpose k chunk-pairs; kaT pair pc holds chunk 2pc on
    # partitions 0:64 and chunk 2pc+1 on 64:128; kbT is materialized on
    # both partition halves so chunk pairs can run as concurrent
    # row-group matmuls (tile_position (0,0) / (64,0)).
    kta = p.kt.tile([128, 2, 128], F32, tag="kta")
    kbd = p.kt.tile([128, NCH, 128], F32, tag="kbd")
    for pc in range(2):
        ps_t = p.ps_tr.tile([128, 128], F32, tag="ps_t")
        nc.tensor.transpose(ps_t[:], p.kraw[:, 0, b, 2 * pc:2 * pc + 2, :],
                            p.ident[:])
        nc.scalar.copy(kta[:, pc, :], ps_t[:])
    for pc in range(2):
        ps_t = p.ps_tr.tile([128, 128], F32, tag="ps_t")
        nc.tensor.transpose(ps_t[:], p.kraw[:, 1, b, 2 * pc:2 * pc + 2, :],
                            p.ident[:])
        nc.scalar.copy(kbd[0:64, 2 * pc, :], ps_t[0:64, :])
        nc.scalar.copy(kbd[64:128, 2 * pc + 1, :], ps_t[64:128, :])
    # fill the missing quadrants (even chunks -> upper half, odd -> lower)
    lo = kbd[0:64, :, :]
    up = kbd[64:128, :, :]
    ev_lo = bass.AP(lo.tensor, lo.offset, [[lo.ap[0][0], 64],
                                           [256, 2], [1, 128]])
    ev_up = bass.AP(up.tensor, up.offset, [[up.ap[0][0], 64],
                                           [256, 2], [1, 128]])
    od_lo = bass.AP(lo.tensor, lo.offset + 128, [[lo.ap[0][0], 64],
                                                 [256, 2], [1, 128]])
    od_up = bass.AP(up.tensor, up.offset + 128, [[up.ap[0][0], 64],
                                                 [256, 2], [1, 128]])
    nc.sync.dma_start(ev_up, ev_lo)
    nc.scalar.dma_start(od_lo, od_up)

    # ---- scores + node_max + argmax per 128-row chunk ----
    nm = p.small.tile([128, NCH], F32, tag="nm")
    nrev = p.small.tile([128, NCH], F32, tag="nrev")
    for mc in range(NCH):
        base = 64 * (mc % 2)
        ps = p.ps_score.tile([128, TH], F32, tag="ps")
        nc.tensor.matmul(ps[:], kta[base:base + 64, mc // 2, :],
                         kbd[base:base + 64, :, :], start=True, stop=True,
                         tile_position=(base, 0))
        nc.vector.tensor_reduce(nm[:, mc:mc + 1], ps[:], axis=AX.X,
                                op=ALU.max)
        if mc == 0:
            # protect first src token: node_max[0] = -inf
            nc.vector.memset(nm[0:1, 0:1], NEG_INF)
        # masked = (ps >= nm) * iota_rev; max -> TH-1 - argmax_first
        masked = p.scr.tile([128, TH], F32, tag="masked")
        nc.vector.scalar_tensor_tensor(masked[:], ps[:], nm[:, mc:mc + 1],
                                       p.iota_rev[:], op0=ALU.is_ge,
                                       op1=ALU.mult)
        nc.vector.tensor_reduce(nrev[:, mc:mc + 1], masked[:], axis=AX.X,
                                op=ALU.max)
    nidx_f = p.small.tile([128, NCH], F32, tag="nidx_f")
    nc.vector.tensor_scalar(nidx_f[:], nrev[:], -1.0, float(TH - 1),
                            op0=ALU.mult, op1=ALU.add)

    # ---- broadcast node_max along partitions: vb[i, j] = v[j] ----
    # transpose nm to [4, 128], expand to block-diagonal [4, 512], then
    # one K=4 matmul with an all-ones lhsT broadcasts it to 128 rows.
    ps_nmT = p.ps_tr.tile([NCH, 128], F32, tag="ps_t")
    nc.tensor.transpose(ps_nmT[:], nm[:, 0:NCH], p.ident[:])
    nmT = p.small.tile([NCH, 128], F32, tag="nmT")
    nc.scalar.copy(nmT[:], ps_nmT[:])
    nap = nmT[:]
    nm_rep = bass.AP(nap.tensor, nap.offset,
                     [[nap.ap[0][0], NCH], [0, NCH], [1, 128]])
    bd = p.small.tile([NCH, TH], F32, tag="bd")
    nc.vector.tensor_tensor(bd[:], nm_rep, p.blockmask[:], op=ALU.mult)
    vb = p.ps_vb.tile([128, TH], F32, tag="vb")
    nc.tensor.matmul(vb[:], p.ones4[:], bd[:], start=True, stop=True)

    # ---- rank[i] = #{v[j] > v[i]} + #{j<i: v[j]==v[i]} ----
    gt_s = p.small.tile([128, NCH], F32, tag="gt_s")
    tie_s = p.small.tile([128, NCH], F32, tag="tie_s")
    for mc in range(NCH):
        junk = p.scr.tile([128, TH], F32, tag="junk")
        nc.vector.tensor_scalar(junk[:], vb[:], nm[:, mc:mc + 1], None,
                                op0=ALU.is_gt, op1=ALU.add,
                                accum_out=gt_s[:, mc:mc + 1])
        junk2 = p.scr.tile([128, TH], F32, tag="junk2")
        nc.vector.scalar_tensor_tensor(junk2[:], vb[:], nm[:, mc:mc + 1],
                                       p.tri[mc][:], op0=ALU.is_equal,
                                       op1=ALU.mult,
                                       accum_out=tie_s[:, mc:mc + 1])
    rank_f = p.small.tile([128, NCH], F32, tag="rank_f")
    nc.vector.tensor_add(rank_f[:], gt_s[:], tie_s[:])

    # ---- int16 casts + wrapped-position transform ----
    # store token of rank r at position (r%16)*32 + r//16, so that the
    # (i%16, i//16)-wrapped DMA index reads become contiguous slices.
    rank16 = p.small.tile([128, NCH], I16, tag="rank16")
    nc.vector.tensor_copy(rank16[:], rank_f[:])
    nidx16 = p.small.tile([128, NCH], I16, tag="nidx16")
    nc.vector.tensor_copy(nidx16[:], nidx_f[:])
    rw1 = p.small.tile([128, NCH], I16, tag="rw1")
    nc.vector.tensor_scalar(rw1[:], rank16[:], 15, 5,
                            op0=ALU.bitwise_and, op1=ALU.logical_shift_left)
    rw2 = p.small.tile([128, NCH], I16, tag="rw2")
    nc.vector.tensor_scalar(rw2[:], rank16[:], 4, None,
                            op0=ALU.logical_shift_right)
    rankw = p.small.tile([128, NCH], I16, tag="rankw")
    nc.vector.tensor_tensor(rankw[:], rw1[:], rw2[:], op=ALU.bitwise_or)

    # ---- local_scatter: perm wrapped (row0) and dstv by rank (row1) ----
    ls_idx = p.idx.tile([16, TH], I16, tag="ls_idx")
    ls_dat = p.idx.tile([16, TH], I16, tag="ls_dat")
    ls_out = p.idx.tile([16, TH], I16, tag="ls_out")
    nc.vector.memset(ls_idx[:, :], -1)
    nc.sync.dma_start(ls_idx[0:1, :], rankw[:, :])
    nc.sync.dma_start(ls_idx[1:2, :], rank16[:, :])
    nc.vector.tensor_copy(ls_dat[0:1, :], p.data_iota[:])
    nc.sync.dma_start(ls_dat[1:2, :], nidx16[:, :])
    nc.gpsimd.local_scatter(ls_out[:], ls_dat[:], ls_idx[:],
                            channels=16, num_elems=TH, num_idxs=TH)

    # ---- gather index tile, replicated to all 8 gpsimd core groups ----
    g_idx = p.ho.tile([128, TH // 16], I16, tag="g_idx")
    bounce = p.dram.tile([TH], I16, tag="bounce")
    nc.sync.dma_start(bounce[:], ls_out[0:1, :])
    bap = bounce[:]
    rep = bass.AP(bap.tensor, bap.offset,
                  [[0, 8], [TH // 16, 16], [1, TH // 16]])
    nc.sync.dma_start(g_idx[:, :], rep)

    return g_idx, ls_out


def _emit_payload(tc, nc, out, x, b, p, g_idx, ls_out):
    # ---- dst-token columns (by rank) + one-hot scatter matrices ----
    dstv16 = p.small.tile([128, 2], I16, tag="dstv16")
    nc.scalar.dma_start(dstv16[:, 0:1], ls_out[1:2, 0:128])
    nc.scalar.dma_start(dstv16[:, 1:2], ls_out[1:2, 128:256])
    dstv_f = p.small.tile([128, 2], F32, tag="dstv_f")
    nc.vector.tensor_copy(dstv_f[:], dstv16[:])
    M = p.mm.tile([128, 2, TH], F16, tag="M")
    for kc in range(2):
        nc.vector.tensor_scalar(M[:, kc, :], p.iota16[:],
                                dstv_f[:, kc:kc + 1], None,
                                op0=ALU.is_equal)
    xb = x[b]                                    # [T, C]
    x_even = xb.rearrange("(t two) c -> two t c", two=2)[0]  # src, stride 2C
    ob = out[b]

    # dst tokens: load all 512 rows (one DMA)
    D = p.d.tile([128, NCH, C], F32, tag="D")
    d_src = bass.AP(xb.tensor, xb.offset + C,
                    [[2 * C, 128], [2 * 128 * C, NCH], [1, C]])
    nc.sync.dma_start(D[:], d_src)

    # unmerged rows (rank 256..511) -> out rows 0..255
    Gu = p.g.tile([128, 2, C], F32, tag="Gu")
    nc.gpsimd.dma_gather(Gu[:], x_even, g_idx[:, 16:32], num_idxs=R,
                         num_idxs_reg=R, elem_size=C, elem_step=2 * C)
    unm_dst = bass.AP(ob.tensor, ob.offset,
                      [[C, 128], [128 * C, 2], [1, C]])
    nc.sync.dma_start(unm_dst, Gu[:])

    # merged rows (rank 0..255), cast to bf16 for the scatter matmul
    Gm = p.g.tile([128, 2, C], F32, tag="Gm")
    nc.gpsimd.dma_gather(Gm[:], x_even, g_idx[:, 0:16], num_idxs=R,
                         num_idxs_reg=R, elem_size=C, elem_step=2 * C)
    Gb = p.g.tile([128, 2, C], F16, tag="Gb")
    nc.scalar.copy(Gb[:], Gm[:])

    # merged rows: out[256+d] = x_odd[d] + sum_{rank q<256, dstv[q]==d} G[q]
    NH = C // 2                                  # 384-column halves
    O = p.o.tile([128, NCH, C], F32, tag="O")
    for dc in range(NCH):
        for nh in range(2):
            S = p.ps_s.tile([128, NH], F32, tag="S")
            for kc in range(2):
                nc.tensor.matmul(S[:], M[:, kc, dc * 128:(dc + 1) * 128],
                                 Gb[:, kc, nh * NH:(nh + 1) * NH],
                                 start=(kc == 0), stop=(kc == 1))
            nc.vector.tensor_add(O[:, dc, nh * NH:(nh + 1) * NH],
                                 D[:, dc, nh * NH:(nh + 1) * NH], S[:])
    o_dst = bass.AP(ob.tensor, ob.offset + NU * C,
                    [[C, 128], [128 * C, NCH], [1, C]])
    nc.scalar.dma_start(o_dst, O[:])

_NC_CACHE = {}


def _get_nc():
    if "nc" not in _NC_CACHE:
        _NC_CACHE["nc"] = build_nc()
    return _NC_CACHE["nc"]


def kernel(x=None, k=None, r=None, _trace=False, **_ignored):
    x = np.ascontiguousarray(np.asarray(x, dtype=np.float32))
    k = np.ascontiguousarray(np.asarray(k, dtype=np.float32))
    rv = int(np.asarray(r)) if r is not None else R
    assert rv == R, f"kernel compiled for r={R}, got r={rv}"
    assert x.shape == (B, T, C) and k.shape == (B, T, CK)

    nc = _get_nc()
    in_maps = [
        {"x": x[i * BL:(i + 1) * BL], "k": k[i * BL:(i + 1) * BL]}
        for i in range(NCORES)
    ]
    res = run_bass_kernel_spmd(nc, in_maps, list(range(NCORES)),
                               trace=_trace)
    outs = [np.asarray(res.results[i]["out"]) for i in range(NCORES)]
    full = np.concatenate(outs, axis=0).astype(np.float32, copy=False)
    if _trace:
        return full, res
    return full


# revision 29
# speedup vs baseline: 1.0066x; 1.0066x over previous
"""BSM (bipartite soft matching) token-merge kernel for Trainium2.

Data-parallel over the batch dim: 64 batch rows are split 8-per-core
across 8 NeuronCores; each core runs an identical Bass program.

Emission is software-pipelined (payload lags index by LAG batches) so
the GPSIMD stream interleaves as ls(0..LAG-1), gather(0), ls(LAG),
gather(1), ... and payload gathers never queue behind index ops that
are still waiting on their inputs.

Per batch:
  index:   scores = a @ b.T (PE fp32); node_max/argmax (DVE fused
           reduces); rank via one-hot comparisons (PE block-diagonal
           broadcast + DVE); perm/dstv via GPSIMD local_scatter.
  payload: dma_gather src rows in rank order (merged/unmerged
           halves); unmerged stored verbatim (fp32); merged summed
           into dst tokens via bf16 one-hot matmul on the PE (one-hot
           side exact; payload side rounds to bf16 on the merged-sum
           term only), added to dst tokens in fp32.
"""

import sys
from contextlib import ExitStack

for _p in ("/root/.axon_site/_ro/trn_rl_repo", "/opt/trn_rl_repo"):
    if _p not in sys.path:
        sys.path.append(_p)

import numpy as np  # noqa: E402

from concourse import bacc, bass, tile  # noqa: E402
from concourse import mybir  # noqa: E402
from concourse.bass_utils import run_bass_kernel_spmd  # noqa: E402

DT = mybir.dt
F32 = DT.float32
BF16 = DT.bfloat16
F16 = DT.float16
I16 = DT.int16
ALU = mybir.AluOpType
AX = mybir.AxisListType

B, T, C, CK, R = 64, 1024, 768, 64, 256
NCORES = 8
BL = B // NCORES          # 8 batch rows per core
TH = T // 2               # 512 source (and dst) tokens
NU = TH - R               # 256 unmerged tokens
NCH = TH // 128           # 4 chunks of 128 source tokens

NEG_INF = -1e30
LAG = 4                   # payload emission lags index by LAG batches


def build_nc(bl: int = BL, debug: bool = False):
    nc = bacc.Bacc("TRN2", target_bir_lowering=False, debug=debug)
    x = nc.dram_tensor("x", [bl, T, C], F32, kind="ExternalInput")
    k = nc.dram_tensor("k", [bl, T, CK], F32, kind="ExternalInput")
    out = nc.dram_tensor("out", [bl, T - R, C], F32, kind="ExternalOutput")

    with tile.TileContext(nc) as tc:
        emit(tc, out.ap(), x.ap(), k.ap(), bl)

    nc.compile()
    return nc


class P:
    """Pool + constant handles shared by the emit phases."""


def emit(tc: tile.TileContext, out: bass.AP, x: bass.AP, k: bass.AP, bl: int):
    nc = tc.nc
    ctx = ExitStack()
    with ctx:
        p = P()
        p.const = ctx.enter_context(tc.tile_pool(name="const", bufs=1))
        p.kall = ctx.enter_context(tc.tile_pool(name="kall", bufs=1))
        p.kt = ctx.enter_context(tc.tile_pool(name="kt", bufs=3))
        p.small = ctx.enter_context(tc.tile_pool(name="small", bufs=3))
        p.scr = ctx.enter_context(tc.tile_pool(name="scr", bufs=3))
        p.idx = ctx.enter_context(tc.tile_pool(name="idx", bufs=6))
        p.mm = ctx.enter_context(tc.tile_pool(name="mm", bufs=2))
        p.ho = ctx.enter_context(tc.tile_pool(name="ho", bufs=LAG + 2))
        p.g = ctx.enter_context(tc.tile_pool(name="g", bufs=3))
        p.d = ctx.enter_context(tc.tile_pool(name="d", bufs=2))
        p.o = ctx.enter_context(tc.tile_pool(name="o", bufs=2))
        p.dram = ctx.enter_context(tc.tile_pool(name="dram", bufs=4,
                                                space="DRAM"))
        p.ps_score = ctx.enter_context(
            tc.tile_pool(name="ps_score", bufs=2, space="PSUM"))
        p.ps_tr = ctx.enter_context(
            tc.tile_pool(name="ps_tr", bufs=2, space="PSUM"))
        p.ps_vb = ctx.enter_context(
            tc.tile_pool(name="ps_vb", bufs=2, space="PSUM"))
        p.ps_s = ctx.enter_context(
            tc.tile_pool(name="ps_s", bufs=2, space="PSUM"))

        # ---- constants ----
        ones_sq = p.const.tile([128, 128], F32)
        nc.vector.memset(ones_sq[:], 1.0)
        p.ident = p.const.tile([128, 128], F32)    # PE transpose identity
        nc.gpsimd.affine_select(
            p.ident[:], ones_sq[:], pattern=[[-1, 128]], base=0,
            channel_multiplier=1, compare_op=ALU.is_equal, fill=0.0)
        p.ones4 = p.const.tile([4, 128], F32)      # lhsT for bcast matmul
        nc.vector.memset(p.ones4[:], 1.0)
        ones_row = p.const.tile([128, TH], F32)
        nc.vector.memset(ones_row[:], 1.0)
        p.iota_bc = p.const.tile([128, TH], F32)   # 0..511 per partition
        nc.gpsimd.iota(p.iota_bc[:], pattern=[[1, TH]], base=0,
                       channel_multiplier=0,
                       allow_small_or_imprecise_dtypes=True)
        p.iota_rev = p.const.tile([128, TH], F32)  # 511..0 per partition
        nc.gpsimd.iota(p.iota_rev[:], pattern=[[-1, TH]], base=TH - 1,
                       channel_multiplier=0,
                       allow_small_or_imprecise_dtypes=True)
        p.tri = []
        for mc in range(NCH):
            t_ = p.const.tile([128, TH], F32, tag=f"tri{mc}")
            # tri[i_local, j] = 1 if j < 128*mc + i_local else 0
            nc.gpsimd.affine_select(
                t_[:], ones_row[:], pattern=[[-1, TH]], base=128 * mc,
                channel_multiplier=1, compare_op=ALU.is_gt, fill=0.0)
            p.tri.append(t_)
        # blockmask[kb, j] = 1 if j // 128 == kb else 0 (rows 0..3)
        bm_iota = p.const.tile([4, TH], F32, tag="bm_iota")
        nc.gpsimd.iota(bm_iota[:], pattern=[[-1, NCH], [0, 128]],
                       base=0, channel_multiplier=1,
                       allow_small_or_imprecise_dtypes=True)
        p.blockmask = p.const.tile([4, TH], F32, tag="bm_sel")
        nc.vector.tensor_scalar(p.blockmask[:], bm_iota[:], 0.0, None,
                                op0=ALU.is_equal)
        # sigma-order token ids: data_iota[0, 4*p + c] = p + 128*c
        p.data_iota = p.const.tile([1, TH], I16)
        nc.gpsimd.iota(p.data_iota[:], pattern=[[1, 128], [128, NCH]],
                       base=0, channel_multiplier=0)
        p.iota16 = p.const.tile([128, TH], F16, tag="iota16")
        nc.gpsimd.iota(p.iota16[:], pattern=[[1, TH]], base=0,
                       channel_multiplier=0,
                       allow_small_or_imprecise_dtypes=True)

        # ---- bulk k load for all local batches ----
        # kall[p, h, b, mc, c] = k[b, 2*(mc*128+p)+h, c]
        p.kraw = p.kall.tile([128, 2, bl, NCH, CK], F32, tag="kraw")
        for h in range(2):
            k_src = bass.AP(k.tensor, k.offset + h * CK,
                            [[2 * CK, 128], [T * CK, bl],
                             [2 * 128 * CK, NCH], [1, CK]])
            nc.sync.dma_start(p.kraw[:, h, :, :, :], k_src)

        # ---- software-pipelined emission ----
        ho = {}
        for step in range(bl + LAG):
            if step < bl:
                ho[step] = _emit_index(tc, nc, step, p)
            if step >= LAG:
                b = step - LAG
                _emit_payload(tc, nc, out, x, b, p, *ho.pop(b))


def _emit_index(tc, nc, b, p):
    # ---- transpose k chunk-pairs; kaT pair pc holds chunk 2pc on
    # partitions 0:64 and chunk 2pc+1 on 64:128; kbT is materialized on
    # both partition halves so chunk pairs can run as concurrent
    # row-group matmuls (tile_position (0,0) / (64,0)).
    kta = p.kt.tile([128, 2, 128], F32, tag="kta")
    kbd = p.kt.tile([128, NCH, 128], F32, tag="kbd")
    for pc in range(2):
        ps_t = p.ps_tr.tile([128, 128], F32, tag="ps_t")
        nc.tensor.transpose(ps_t[:], p.kraw[:, 0, b, 2 * pc:2 * pc + 2, :],
                            p.ident[:])
        nc.scalar.copy(kta[:, pc, :], ps_t[:])
    for pc in range(2):
        ps_t = p.ps_tr.tile([128, 128], F32, tag="ps_t")
        nc.tensor.transpose(ps_t[:], p.kraw[:, 1, b, 2 * pc:2 * pc + 2, :],
                            p.ident[:])
        nc.scalar.copy(kbd[0:64, 2 * pc, :], ps_t[0:64, :])
        nc.scalar.copy(kbd[64:128, 2 * pc + 1, :], ps_t[64:128, :])
    # fill the missing quadrants (even chunks -> upper half, odd -> lower)
    lo = kbd[0:64, :, :]
    up = kbd[64:128, :, :]
    ev_lo = bass.AP(lo.tensor, lo.offset, [[lo.ap[0][0], 64],
                                           [256, 2], [1, 128]])
    ev_up = bass.AP(up.tensor, up.offset, [[up.ap[0][0], 64],
                                           [256, 2], [1, 128]])
    od_lo = bass.AP(lo.tensor, lo.offset + 128, [[lo.ap[0][0], 64],
                                                 [256, 2], [1, 128]])
    od_up = bass.AP(up.tensor, up.offset + 128, [[up.ap[0][0], 64],
                                                 [256, 2], [1, 128]])
    nc.sync.dma_start(ev_up, ev_lo)
    nc.scalar.dma_start(od_lo, od_up)

    # ---- scores + node_max + argmax per 128-row chunk ----
    nm = p.small.tile([128, NCH], F32, tag="nm")
    nrev = p.small.tile([128, NCH], F32, tag="nrev")
    for mc in range(NCH):
        base = 64 * (mc % 2)
        ps = p.ps_score.tile([128, TH], F32, tag="ps")
        nc.tensor.matmul(ps[:], kta[base:base + 64, mc // 2, :],
                         kbd[base:base + 64, :, :], start=True, stop=True,
                         tile_position=(base, 0))
        nc.vector.tensor_reduce(nm[:, mc:mc + 1], ps[:], axis=AX.X,
                                op=ALU.max)
        if mc == 0:
            # protect first src token: node_max[0] = -inf
            nc.vector.memset(nm[0:1, 0:1], NEG_INF)
        # masked = (ps >= nm) * iota_rev; max -> TH-1 - argmax_first
        masked = p.scr.tile([128, TH], F32, tag="masked")
        nc.vector.scalar_tensor_tensor(masked[:], ps[:], nm[:, mc:mc + 1],
                                       p.iota_rev[:], op0=ALU.is_ge,
                                       op1=ALU.mult)
        nc.vector.tensor_reduce(nrev[:, mc:mc + 1], masked[:], axis=AX.X,
                                op=ALU.max)
    nidx_f = p.small.tile([128, NCH], F32, tag="nidx_f")
    nc.vector.tensor_scalar(nidx_f[:], nrev[:], -1.0, float(TH - 1),
                            op0=ALU.mult, op1=ALU.add)

    # ---- broadcast node_max along partitions: vb[i, j] = v[j] ----
    # transpose nm to [4, 128], expand to block-diagonal [4, 512], then
    # one K=4 matmul with an all-ones lhsT broadcasts it to 128 rows.
    ps_nmT = p.ps_tr.tile([NCH, 128], F32, tag="ps_t")
    nc.tensor.transpose(ps_nmT[:], nm[:, 0:NCH], p.ident[:])
    nmT = p.small.tile([NCH, 128], F32, tag="nmT")
    nc.scalar.copy(nmT[:], ps_nmT[:])
    nap = nmT[:]
    nm_rep = bass.AP(nap.tensor, nap.offset,
                     [[nap.ap[0][0], NCH], [0, NCH], [1, 128]])
    bd = p.small.tile([NCH, TH], F32, tag="bd")
    nc.vector.tensor_tensor(bd[:], nm_rep, p.blockmask[:], op=ALU.mult)
    vb = p.ps_vb.tile([128, TH], F32, tag="vb")
    nc.tensor.matmul(vb[:], p.ones4[:], bd[:], start=True, stop=True)

    # ---- rank[i] = #{v[j] > v[i]} + #{j<i: v[j]==v[i]} ----
    gt_s = p.small.tile([128, NCH], F32, tag="gt_s")
    tie_s = p.small.tile([128, NCH], F32, tag="tie_s")
    for mc in range(NCH):
        junk = p.scr.tile([128, TH], F32, tag="junk")
        nc.vector.tensor_scalar(junk[:], vb[:], nm[:, mc:mc + 1], None,
                                op0=ALU.is_gt, op1=ALU.add,
                                accum_out=gt_s[:, mc:mc + 1])
        junk2 = p.scr.tile([128, TH], F32, tag="junk2")
        nc.vector.scalar_tensor_tensor(junk2[:], vb[:], nm[:, mc:mc + 1],
                                       p.tri[mc][:], op0=ALU.is_equal,
                                       op1=ALU.mult,
                                       accum_out=tie_s[:, mc:mc + 1])
    rank_f = p.small.tile([128, NCH], F32, tag="rank_f")
    nc.vector.tensor_add(rank_f[:], gt_s[:], tie_s[:])

    # ---- int16 casts + wrapped-position transform ----
    # store token of rank r at position (r%16)*32 + r//16, so that the
    # (i%16, i//16)-wrapped DMA index reads become contiguous slices.
    rank16 = p.small.tile([128, NCH], I16, tag="rank16")
    nc.vector.tensor_copy(rank16[:], rank_f[:])
    nidx16 = p.small.tile([128, NCH], I16, tag="nidx16")
    nc.vector.tensor_copy(nidx16[:], nidx_f[:])
    rw1 = p.small.tile([128, NCH], I16, tag="rw1")
    nc.vector.tensor_scalar(rw1[:], rank16[:], 15, 5,
                            op0=ALU.bitwise_and, op1=ALU.logical_shift_left)
    rw2 = p.small.tile([128, NCH], I16, tag="rw2")
    nc.vector.tensor_scalar(rw2[:], rank16[:], 4, None,
                            op0=ALU.logical_shift_right)
    rankw = p.small.tile([128, NCH], I16, tag="rankw")
    nc.vector.tensor_tensor(rankw[:], rw1[:], rw2[:], op=ALU.bitwise_or)

    # ---- local_scatter: perm wrapped (row0) and dstv by rank (row1) ----
    ls_idx = p.idx.tile([16, TH], I16, tag="ls_idx")
    ls_dat = p.idx.tile([16, TH], I16, tag="ls_dat")
    ls_out = p.idx.tile([16, TH], I16, tag="ls_out")
    nc.vector.memset(ls_idx[:, :], -1)
    nc.sync.dma_start(ls_idx[0:1, :], rankw[:, :])
    nc.sync.dma_start(ls_idx[1:2, :], rank16[:, :])
    nc.vector.tensor_copy(ls_dat[0:1, :], p.data_iota[:])
    nc.sync.dma_start(ls_dat[1:2, :], nidx16[:, :])
    nc.gpsimd.local_scatter(ls_out[:], ls_dat[:], ls_idx[:],
                            channels=16, num_elems=TH, num_idxs=TH)

    # ---- gather index tile, replicated to all 8 gpsimd core groups ----
    g_idx = p.ho.tile([128, TH // 16], I16, tag="g_idx")
    bounce = p.dram.tile([TH], I16, tag="bounce")
    nc.sync.dma_start(bounce[:], ls_out[0:1, :])
    bap = bounce[:]
    rep = bass.AP(bap.tensor, bap.offset,
                  [[0, 8], [TH // 16, 16], [1, TH // 16]])
    nc.sync.dma_start(g_idx[:, :], rep)

    return g_idx, ls_out


def _emit_payload(tc, nc, out, x, b, p, g_idx, ls_out):
    # ---- dst-token columns (by rank) + one-hot scatter matrices ----
    dstv16 = p.small.tile([128, 2], I16, tag="dstv16")
    nc.scalar.dma_start(dstv16[:, 0:1], ls_out[1:2, 0:128])
    nc.scalar.dma_start(dstv16[:, 1:2], ls_out[1:2, 128:256])
    dstv_f = p.small.tile([128, 2], F32, tag="dstv_f")
    nc.vector.tensor_copy(dstv_f[:], dstv16[:])
    M = p.mm.tile([128, 2, TH], F16, tag="M")
    for kc in range(2):
        nc.vector.tensor_scalar(M[:, kc, :], p.iota16[:],
                                dstv_f[:, kc:kc + 1], None,
                                op0=ALU.is_equal)
    xb = x[b]                                    # [T, C]
    x_even = xb.rearrange("(t two) c -> two t c", two=2)[0]  # src, stride 2C
    ob = out[b]

    # dst tokens: load all 512 rows (one DMA)
    D = p.d.tile([128, NCH, C], F32, tag="D")
    d_src = bass.AP(xb.tensor, xb.offset + C,
                    [[2 * C, 128], [2 * 128 * C, NCH], [1, C]])
    nc.sync.dma_start(D[:], d_src)

    # unmerged rows (rank 256..511) -> out rows 0..255
    Gu = p.g.tile([128, 2, C], F32, tag="Gu")
    nc.gpsimd.dma_gather(Gu[:], x_even, g_idx[:, 16:32], num_idxs=R,
                         num_idxs_reg=R, elem_size=C, elem_step=2 * C)
    unm_dst = bass.AP(ob.tensor, ob.offset,
                      [[C, 128], [128 * C, 2], [1, C]])
    nc.sync.dma_start(unm_dst, Gu[:])

    # merged rows (rank 0..255), cast to bf16 for the scatter matmul
    Gm = p.g.tile([128, 2, C], F32, tag="Gm")
    nc.gpsimd.dma_gather(Gm[:], x_even, g_idx[:, 0:16], num_idxs=R,
                         num_idxs_reg=R, elem_size=C, elem_step=2 * C)
    Gb = p.g.tile([128, 2, C], F16, tag="Gb")
    nc.scalar.copy(Gb[:], Gm[:])

    # merged rows: out[256+d] = x_odd[d] + sum_{rank q<256, dstv[q]==d} G[q]
    NH = C // 2                                  # 384-column halves
    O = p.o.tile([128, NCH, C], F32, tag="O")
    for dc in range(NCH):
        for nh in range(2):
            S = p.ps_s.tile([128, NH], F32, tag="S")
            for kc in range(2):
                nc.tensor.matmul(S[:], M[:, kc, dc * 128:(dc + 1) * 128],
                                 Gb[:, kc, nh * NH:(nh + 1) * NH],
                                 start=(kc == 0), stop=(kc == 1))
            nc.vector.tensor_add(O[:, dc, nh * NH:(nh + 1) * NH],
                                 D[:, dc, nh * NH:(nh + 1) * NH], S[:])
    o_dst = bass.AP(ob.tensor, ob.offset + NU * C,
                    [[C, 128], [128 * C, NCH], [1, C]])
    nc.scalar.dma_start(o_dst, O[:])

_NC_CACHE = {}


def _get_nc():
    if "nc" not in _NC_CACHE:
        _NC_CACHE["nc"] = build_nc()
    return _NC_CACHE["nc"]


def kernel(x=None, k=None, r=None, _trace=False, **_ignored):
    x = np.ascontiguousarray(np.asarray(x, dtype=np.float32))
    k = np.ascontiguousarray(np.asarray(k, dtype=np.float32))
    rv = int(np.asarray(r)) if r is not None else R
    assert rv == R, f"kernel compiled for r={R}, got r={rv}"
    assert x.shape == (B, T, C) and k.shape == (B, T, CK)

    nc = _get_nc()
    in_maps = [
        {"x": x[i * BL:(i + 1) * BL], "k": k[i * BL:(i + 1) * BL]}
        for i in range(NCORES)
    ]
    res = run_bass_kernel_spmd(nc, in_maps, list(range(NCORES)),
                               trace=_trace)
    outs = [np.asarray(res.results[i]["out"]) for i in range(NCORES)]
    full = np.concatenate(outs, axis=0).astype(np.float32, copy=False)
    if _trace:
        return full, res
    return full
